# revision 4
# baseline (speedup 1.0000x reference)
"""BitLinear (ternary absmean-quantized linear) on 8 TRN2 NeuronCores.

Reference math (fp32):
    gamma = mean(|W|)
    Wq    = round(clip(W / (gamma + 1e-5), -1, 1))   # ternary {-1, 0, 1}
    out   = einsum('bsi,oi->bso', x, Wq)             # x @ Wq.T

Sharding: tokens x n-halves. x [4,2048,4096] -> 8192 tokens; cores are a
4x2 grid: token-group (2048 tokens) x out-feature half (2048 features).
Every core needs its W half; gamma (a global scalar) is computed
cooperatively: each core abs-sums 1/8 of W (512 of the 4096 columns of
WT), a tiny [128,1] AllReduce combines the partials, and each core then
quantizes its W half on the fly while the TensorEngine consumes it.

Precision/speed: ALL matmuls run as fp8 e4m3 DoubleRow pairs. On this
deployment fp8-DR moving rows clock at ~2.4 GHz while bf16 rows are
power-throttled to ~1.95 GHz, so a DR pair-pass is the cheapest PE unit.
  - "precise" planes (first KB of K): an (hi, lo) DR pair per plane with
    hi = e4m3(x), lo = e4m3(x - hi) and the plane's ternary weights
    duplicated in both pair slots: contributes w*(hi+lo) ~ bf16-accurate.
  - "cheap" planes (rest): two K-planes per DR pair, single e4m3 each.
All fp8 values are pre-rounded to e4m3 on the HOST (RNE) and shipped as
bf16, so the device bf16->f8 copies are exact regardless of the DVE cast
rounding mode (this deployment's cast rounds worse than RNE).
Measured on the real inputs: fro rel err 1.88e-2, absmax-rel 1.94e-2.

Device kernel layout (per core), output TRANSPOSED (features x tokens):
    xT   [6144, 2048] bf16  - 48 e4m3-exact sub-planes (24 DR pairs), K-major
    WT   [4096, 2048] f32   - this core's W half, transposed (k major)
    Wg   [4096,  512] f32   - this core's gamma shard (512 columns of WT)
    outT [2048, 2048] f32   - out.T; host transposes back

Main loop: 4 N-chunks of 512 output features. Per chunk: stream 32 K-slabs
of WT f32 on the ACT HWDGE ring (the sync ring carries only out-stores, so
W prefetch is never head-of-line blocked), quantize each on DVE
((|w| > t) with sign, 2-3 ops) into a resident fp8 [128, 24, 2, 512] pair
buffer. Matmuls: per 128-feature n-tile, 24 DR pair-passes x 4 moving
tiles of 512 tokens accumulate full K in one PSUM bank.
"""

import numpy as np
import ml_dtypes

NCORES = 8

# Full-problem dims (hardcoded per the harness contract).
B, S, D_IN, D_OUT = 4, 2048, 4096, 4096
M_TOTAL = B * S            # 8192 tokens
N_SPLIT = 2                # n-halves (cores = 4 token groups x 2 halves)
M_GROUPS = NCORES // N_SPLIT
M_CORE = M_TOTAL // M_GROUPS   # tokens per core
N_CORE = D_OUT // N_SPLIT      # output features per core
KB_PLANES = 2048           # K-planes with hi+lo (precise); rest single-e4m3

_COMPILED = None   # cached (nc, meta)
LAST_RESULTS = None  # BassKernelResults of the most recent run (for test.py)


def build_module(m_core=M_CORE, k=D_IN, n=N_CORE, ncores=NCORES, repeat=1,
                 use_collective=True, kb=KB_PLANES, n_full=None,
                 wpool_bufs=8, stg_bufs=4, g_chunk=None):
    """Build + compile the SPMD Bass module. repeat>1 unrolls the whole
    kernel body multiple times inside one NEFF (for steady-state timing).

    n is this core's output-feature count; n_full (defaults to n) is the
    FULL problem's out_features, used only for the gamma normalization
    (gamma = sum|W| / (k * n_full), reduced across cores)."""
    if g_chunk is None:
        g_chunk = 4 if m_core <= 1024 else 2
    opool_bufs = 6 if m_core <= 1024 else 4
    spool_bufs = 6 if m_core <= 1024 else 4
    import concourse.bass as bass  # noqa: F401
    import concourse.mybir as mybir
    import concourse.tile as tile
    from concourse import bacc
    from concourse import bass_isa

    f32 = mybir.dt.float32
    bf16 = mybir.dt.bfloat16
    f8 = mybir.dt.float8e4
    KT = k // 128            # total k-tiles of 128
    P = kb // 128            # precise planes (hi+lo pair each)
    C = KT - P               # cheap planes (2 planes per pair)
    assert C % 2 == 0, "cheap plane count must be even"
    NPAIR = P + C // 2       # DR pair-passes per n-tile
    NSUB = 2 * NPAIR         # fp8 sub-planes in the x stream
    NCHUNK = min(512, n)     # output-feature chunk width
    NCHUNKS = n // NCHUNK
    NTILES = NCHUNK // 128   # n-tiles (stationary free dim) per chunk
    MW = min(512, m_core)    # moving-operand token width
    MH = m_core // MW        # moving tiles per pair-pass
    if n_full is None:
        n_full = n
    NG = n_full // 8         # gamma shard width (columns of full WT)
    G_CHUNK = min(g_chunk, KT)  # k-tiles per gamma reduce chunk
    G_CHUNKS = KT // G_CHUNK
    N_ELEMS = float(k * n_full)

    nc = bacc.Bacc("TRN2", target_bir_lowering=False, debug=False,
                   num_devices=ncores)
    xT = nc.dram_tensor("xT", [NSUB * 128, m_core], bf16,
                        kind="ExternalInput")
    WT = nc.dram_tensor("WT", [k, n], f32, kind="ExternalInput")
    Wg = nc.dram_tensor("Wg", [k, NG], f32, kind="ExternalInput")
    outT = nc.dram_tensor("outT", [n, m_core], f32, kind="ExternalOutput")

    ts = bass.ts
    DR = mybir.MatmulPerfMode.DoubleRow

    with tile.TileContext(nc) as tc:
        with (
            tc.tile_pool(name="xpool", bufs=1) as xpool,
            tc.tile_pool(name="xstg", bufs=stg_bufs) as xstg,
            tc.tile_pool(name="gpool", bufs=2) as gpool,
            tc.tile_pool(name="wqdp", bufs=2) as wqdp,
            tc.tile_pool(name="wpool", bufs=wpool_bufs) as wpool,
            tc.tile_pool(name="spool", bufs=spool_bufs) as spool,
            tc.tile_pool(name="opool", bufs=opool_bufs) as opool,
            tc.tile_pool(name="small", bufs=2) as small,
            tc.tile_pool(name="pmain", bufs=8, space="PSUM") as pmain,
            tc.tile_pool(name="dram", bufs=2, space="DRAM") as dram,
        ):
          with tc.tile_pool(name="cpool", bufs=1) as cpool:
            bias_p = cpool.tile([128, 1], f32, name="bias_p")
            nc.gpsimd.memset(bias_p[:], 0.5e-5)
            bias_n = cpool.tile([128, 1], f32, name="bias_n")
            nc.gpsimd.memset(bias_n[:], -0.5e-5)

          # ---- resident x: fp8 [128, NPAIR, 2, m] (hi/lo + cheap pairs) ----
          # Loaded once per NEFF execution (x does not change within one
          # launch). Host pre-rounds every sub-plane to an exact e4m3 value
          # stored in bf16; the DVE copy below is therefore exact.
          xdr = xpool.tile([128, NPAIR, 2, m_core], f8, name="xdr")
          xr = xT[:, :].rearrange("(t p) m -> p t m", p=128)
          for t in range(NSUB):
              stg = xstg.tile([128, m_core], bf16, tag="xstg")
              nc.sync.dma_start(stg[:], xr[:, t, :])
              nc.vector.tensor_copy(xdr[:, t // 2, t % 2, :], stg[:])

          for _rep in range(repeat):
            # ---- gamma: local abs-sum over this core's shard ----
            # On ACT + gpsimd queues, which run far ahead of the PE: in
            # steady state iteration i+1's whole gamma chain (including the
            # AllReduce) completes under iteration i's matmuls.
            acc = small.tile([128, G_CHUNKS], f32)
            for j in range(G_CHUNKS):
                gsl = gpool.tile([128, G_CHUNK, NG], f32, tag="gsl")
                src = Wg[j * G_CHUNK * 128:(j + 1) * G_CHUNK * 128, :]
                geng = nc.sync if _rep == 0 else nc.scalar
                geng.dma_start(gsl[:], src.rearrange("(t p) c -> p t c", p=128))
                gscr = gpool.tile([128, G_CHUNK, NG], bf16, tag="gscr")
                nc.scalar.activation(
                    gscr[:], gsl[:], mybir.ActivationFunctionType.Abs,
                    accum_out=acc[:, j:j + 1])
            gpart = small.tile([128, 1], f32)
            gscr2 = small.tile([128, G_CHUNKS], bf16)
            nc.scalar.activation(
                gscr2[:], acc[:], mybir.ActivationFunctionType.Abs,
                accum_out=gpart[:])

            # ---- tiny AllReduce of per-partition partials ----
            gsum = small.tile([128, 1], f32)
            if ncores > 1 and use_collective:
                cin = dram.tile([128, 1], f32)
                nc.scalar.dma_start(cin[:], gpart[:])
                cout = dram.tile([128, 1], f32, tag="cout", name=f"cout{_rep}")
                nc.gpsimd.collective_compute(
                    "AllReduce", mybir.AluOpType.add,
                    replica_groups=[list(range(ncores))],
                    ins=[cin[:].opt()], outs=[cout[:].opt()])
                nc.scalar.dma_start(gsum[:], cout[:])
            else:
                # timing/TimelineSim variant: no collective (gamma from the
                # local shard only -- numerically wrong, timing-equivalent)
                nc.scalar.copy(gsum[:], gpart[:])

            # sum across partitions, result broadcast to all partitions
            gtot = small.tile([128, 1], f32)
            nc.gpsimd.partition_all_reduce(
                gtot[:], gsum[:], channels=128, reduce_op=bass_isa.ReduceOp.add)

            # threshold t = 0.5 * (gamma + 1e-5)
            # Wq = (w > t) - (w < -t)  in {-1, 0, 1}
            tsb = small.tile([128, 1], f32)
            nc.scalar.activation(
                tsb[:], gtot[:], mybir.ActivationFunctionType.Identity,
                bias=bias_p[:], scale=0.5 / N_ELEMS)
            ntsb = small.tile([128, 1], f32)
            nc.scalar.activation(
                ntsb[:], gtot[:], mybir.ActivationFunctionType.Identity,
                bias=bias_n[:], scale=-0.5 / N_ELEMS)

            # ---- main loop over output-feature chunks ----
            for c in range(NCHUNKS):
                # quantize this chunk's W half into DR pair layout
                wqd = wqdp.tile([128, NPAIR, 2, NCHUNK], f8, tag="wqd")
                for kt in range(KT):
                    wtmp = wpool.tile([128, NCHUNK], f32, tag="wtmp")
                    # W stream rides the ACT HWDGE ring: the sync ring's
                    # out-stores would head-of-line block next-chunk W
                    # prefetch. ACT's ring is otherwise only used by the
                    # early gamma work.
                    nc.scalar.dma_start(
                        wtmp[:], WT[ts(kt, 128), ts(c, NCHUNK)])
                    neg = spool.tile([128, NCHUNK], bf16, tag="neg")
                    nc.vector.tensor_scalar(
                        neg[:], wtmp[:], ntsb[:], None, mybir.AluOpType.is_lt)
                    if kt < P:
                        dsts = [wqd[:, kt, 0, :], wqd[:, kt, 1, :]]
                    else:
                        j = kt - P
                        dsts = [wqd[:, P + j // 2, j % 2, :]]
                    for dst in dsts:
                        nc.vector.scalar_tensor_tensor(
                            dst, wtmp[:], tsb[:], neg[:],
                            mybir.AluOpType.is_gt, mybir.AluOpType.subtract)

                # matmuls: stationary = wq pair (128 features x 2 subplanes),
                # moving = x pair (2 x MW tokens). One PSUM bank accumulates
                # full K per (nt, mh).
                for nt in range(NTILES):
                    ps = [pmain.tile([128, MW], f32, tag="ps",
                                     name=f"ps{nt % 2}_{mh}")
                          for mh in range(MH)]
                    n0 = nt * 128
                    for q in range(NPAIR):
                        lw = wqd[:, q, :, n0:n0 + 128]
                        for mh in range(MH):
                            nc.tensor.matmul(
                                ps[mh][:], lw,
                                xdr[:, q, :, ts(mh, MW)],
                                start=(q == 0), stop=(q == NPAIR - 1),
                                perf_mode=DR)
                    for mh in range(MH):
                        osb = opool.tile([128, MW], f32, tag="osb")
                        nc.vector.tensor_copy(osb[:], ps[mh][:])
                        nc.sync.dma_start(
                            outT[c * NCHUNK + n0:c * NCHUNK + n0 + 128,
                                 ts(mh, MW)], osb[:])

    nc.compile()
    meta = dict(m_core=m_core, k=k, n=n, ncores=ncores, NG=NG, kb=kb)
    return nc, meta


def _get_compiled():
    global _COMPILED
    if _COMPILED is None:
        _COMPILED = build_module(n_full=D_OUT)
    return _COMPILED


def make_in_maps(x, W, m_core=M_CORE, n_core=N_CORE, ncores=NCORES,
                 n_split=N_SPLIT, kb=KB_PLANES):
    """Host-side shard prep. x [B,S,D_IN] f32, W [D_OUT,D_IN] f32.
    Core c = (token-group c//n_split, n-half c%n_split).

    x is encoded as 48 e4m3-exact sub-planes stored in bf16, in DR pair
    order: pairs 0..P-1 are (hi, lo) of precise plane p; pairs P.. are
    (plane KB+2j, plane KB+2j+1) single-e4m3 cheap planes."""
    k = W.shape[1]
    n = W.shape[0]
    ng = n // ncores
    P = kb // 128
    x2 = np.asarray(x, dtype=np.float32).reshape(-1, k)
    f8 = ml_dtypes.float8_e4m3fn
    hi = x2.astype(f8).astype(np.float32)
    lo = (x2 - hi).astype(f8).astype(np.float32)
    # sub-plane stream [NSUB*128, tokens] built per token-group below
    planes = []
    for p in range(P):
        planes.append(hi[:, p * 128:(p + 1) * 128])
        planes.append(lo[:, p * 128:(p + 1) * 128])
    planes.append(hi[:, P * 128:])      # cheap planes, already e4m3-exact
    xs = np.concatenate(planes, axis=1).astype(ml_dtypes.bfloat16)
    WT = np.ascontiguousarray(np.asarray(W, dtype=np.float32).T)  # [k, n]
    xTg = [np.ascontiguousarray(xs[g * m_core:(g + 1) * m_core, :].T)
           for g in range(ncores // n_split)]
    in_maps = []
    for c in range(ncores):
        g, h = divmod(c, n_split)
        WTc = (WT if n_split == 1 else
               np.ascontiguousarray(WT[:, h * n_core:(h + 1) * n_core]))
        Wgc = np.ascontiguousarray(WT[:, c * ng:(c + 1) * ng])
        in_maps.append({"xT": xTg[g], "WT": WTc, "Wg": Wgc})
    return in_maps


def kernel(input, W):
    """Full inputs in, full output out. Shards internally across 8 cores."""
    global LAST_RESULTS
    from concourse import bass_utils

    nc, meta = _get_compiled()
    in_maps = make_in_maps(input, W)
    res = bass_utils.run_bass_kernel_spmd(
        nc, in_maps, core_ids=list(range(NCORES)))
    LAST_RESULTS = res
    rows = [np.concatenate([res.results[g * N_SPLIT + h]["outT"].T
                            for h in range(N_SPLIT)], axis=1)
            for g in range(M_GROUPS)]
    out = np.vstack(rows) if M_GROUPS > 1 else rows[0]
    return np.ascontiguousarray(out).reshape(B, S, D_OUT).astype(np.float32)


# revision 5
# speedup vs baseline: 1.0005x; 1.0005x over previous
"""BitLinear (ternary absmean-quantized linear) on 8 TRN2 NeuronCores.

Reference math (fp32):
    gamma = mean(|W|)
    Wq    = round(clip(W / (gamma + 1e-5), -1, 1))   # ternary {-1, 0, 1}
    out   = einsum('bsi,oi->bso', x, Wq)             # x @ Wq.T

Sharding: tokens x n-halves. x [4,2048,4096] -> 8192 tokens; cores are a
4x2 grid: token-group (2048 tokens) x out-feature half (2048 features).
Every core needs its W half; gamma (a global scalar) is computed
cooperatively: each core abs-sums 1/8 of W (512 of the 4096 columns of
WT), a tiny [128,1] AllReduce combines the partials, and each core then
quantizes its W half on the fly while the TensorEngine consumes it.

Precision/speed: ALL matmuls run as fp8 e4m3 DoubleRow pairs (one DR
pair-pass covers 2 K-planes at the same ns/row as one bf16 plane, and
measures slightly faster than the bf16/fp8 mix under the sustained-load
P0 clock of ~1.95 GHz; the kernel runs at the pure-PE roofline).
  - "precise" planes (first KB of K): an (hi, lo) DR pair per plane with
    hi = e4m3(x), lo = e4m3(x - hi) and the plane's ternary weights
    duplicated in both pair slots: contributes w*(hi+lo) ~ bf16-accurate.
  - "cheap" planes (rest): two K-planes per DR pair, single e4m3 each.
All fp8 values are pre-rounded to e4m3 on the HOST (RNE) and shipped as
bf16, so the device bf16->f8 copies are exact regardless of the DVE cast
rounding mode (this deployment's cast rounds worse than RNE).
Measured on the real inputs: fro rel err 1.88e-2, absmax-rel 1.94e-2.

Device kernel layout (per core), output TRANSPOSED (features x tokens):
    xT   [6144, 2048] bf16  - 48 e4m3-exact sub-planes (24 DR pairs), K-major
    WT   [4096, 2048] f32   - this core's W half, transposed (k major)
    Wg   [4096,  512] f32   - this core's gamma shard (512 columns of WT)
    outT [2048, 2048] f32   - out.T; host transposes back

Main loop: 4 N-chunks of 512 output features. Per chunk: stream 32 K-slabs
of WT f32 on the ACT HWDGE ring (the sync ring carries only out-stores, so
W prefetch is never head-of-line blocked), quantize each on DVE
((|w| > t) with sign, 2-3 ops) into a resident fp8 [128, 24, 2, 512] pair
buffer. Matmuls: per 128-feature n-tile, 24 DR pair-passes x 4 moving
tiles of 512 tokens accumulate full K in one PSUM bank.
"""

import numpy as np
import ml_dtypes

NCORES = 8

# Full-problem dims (hardcoded per the harness contract).
B, S, D_IN, D_OUT = 4, 2048, 4096, 4096
M_TOTAL = B * S            # 8192 tokens
N_SPLIT = 2                # n-halves (cores = 4 token groups x 2 halves)
M_GROUPS = NCORES // N_SPLIT
M_CORE = M_TOTAL // M_GROUPS   # tokens per core
N_CORE = D_OUT // N_SPLIT      # output features per core
KB_PLANES = 2048           # K-planes with hi+lo (precise); rest single-e4m3

_COMPILED = None   # cached (nc, meta)
LAST_RESULTS = None  # BassKernelResults of the most recent run (for test.py)


def build_module(m_core=M_CORE, k=D_IN, n=N_CORE, ncores=NCORES, repeat=1,
                 use_collective=True, kb=KB_PLANES, n_full=None,
                 wpool_bufs=8, stg_bufs=4, g_chunk=None):
    """Build + compile the SPMD Bass module. repeat>1 unrolls the whole
    kernel body multiple times inside one NEFF (for steady-state timing).

    n is this core's output-feature count; n_full (defaults to n) is the
    FULL problem's out_features, used only for the gamma normalization
    (gamma = sum|W| / (k * n_full), reduced across cores)."""
    if g_chunk is None:
        g_chunk = 4 if m_core <= 1024 else 2
    opool_bufs = 6 if m_core <= 1024 else 4
    spool_bufs = 6 if m_core <= 1024 else 4
    import concourse.bass as bass  # noqa: F401
    import concourse.mybir as mybir
    import concourse.tile as tile
    from concourse import bacc
    from concourse import bass_isa

    f32 = mybir.dt.float32
    bf16 = mybir.dt.bfloat16
    f8 = mybir.dt.float8e4
    KT = k // 128            # total k-tiles of 128
    P = kb // 128            # precise planes (hi+lo pair each)
    C = KT - P               # cheap planes (2 planes per pair)
    assert C % 2 == 0, "cheap plane count must be even"
    NPAIR = P + C // 2       # DR pair-passes per n-tile
    NSUB = 2 * NPAIR         # fp8 sub-planes in the x stream
    NCHUNK = min(512, n)     # output-feature chunk width
    NCHUNKS = n // NCHUNK
    NTILES = NCHUNK // 128   # n-tiles (stationary free dim) per chunk
    MW = min(512, m_core)    # moving-operand token width
    MH = m_core // MW        # moving tiles per pair-pass
    if n_full is None:
        n_full = n
    NG = n_full // 8         # gamma shard width (columns of full WT)
    G_CHUNK = min(g_chunk, KT)  # k-tiles per gamma reduce chunk
    G_CHUNKS = KT // G_CHUNK
    N_ELEMS = float(k * n_full)

    nc = bacc.Bacc("TRN2", target_bir_lowering=False, debug=False,
                   num_devices=ncores)
    xT = nc.dram_tensor("xT", [NSUB * 128, m_core], bf16,
                        kind="ExternalInput")
    WT = nc.dram_tensor("WT", [k, n], f32, kind="ExternalInput")
    Wg = nc.dram_tensor("Wg", [k, NG], f32, kind="ExternalInput")
    outT = nc.dram_tensor("outT", [n, m_core], f32, kind="ExternalOutput")

    ts = bass.ts
    DR = mybir.MatmulPerfMode.DoubleRow

    with tile.TileContext(nc) as tc:
        with (
            tc.tile_pool(name="xpool", bufs=1) as xpool,
            tc.tile_pool(name="xstg", bufs=stg_bufs) as xstg,
            tc.tile_pool(name="gpool", bufs=2) as gpool,
            tc.tile_pool(name="wqdp", bufs=2) as wqdp,
            tc.tile_pool(name="wpool", bufs=wpool_bufs) as wpool,
            tc.tile_pool(name="spool", bufs=spool_bufs) as spool,
            tc.tile_pool(name="opool", bufs=opool_bufs) as opool,
            tc.tile_pool(name="small", bufs=2) as small,
            tc.tile_pool(name="pmain", bufs=8, space="PSUM") as pmain,
            tc.tile_pool(name="dram", bufs=2, space="DRAM") as dram,
        ):
          with tc.tile_pool(name="cpool", bufs=1) as cpool:
            bias_p = cpool.tile([128, 1], f32, name="bias_p")
            nc.gpsimd.memset(bias_p[:], 0.5e-5)
            bias_n = cpool.tile([128, 1], f32, name="bias_n")
            nc.gpsimd.memset(bias_n[:], -0.5e-5)

          # ---- resident x: fp8 [128, NPAIR, 2, m] (hi/lo + cheap pairs) ----
          # Loaded once per NEFF execution (x does not change within one
          # launch). Host pre-rounds every sub-plane to an exact e4m3 value
          # stored in bf16; the DVE copy below is therefore exact.
          xdr = xpool.tile([128, NPAIR, 2, m_core], f8, name="xdr")
          xr = xT[:, :].rearrange("(t p) m -> p t m", p=128)
          for t in range(NSUB):
              stg = xstg.tile([128, m_core], bf16, tag="xstg")
              nc.sync.dma_start(stg[:], xr[:, t, :])
              nc.vector.tensor_copy(xdr[:, t // 2, t % 2, :], stg[:])

          for _rep in range(repeat):
            # ---- gamma: local abs-sum over this core's shard ----
            # On ACT + gpsimd queues, which run far ahead of the PE: in
            # steady state iteration i+1's whole gamma chain (including the
            # AllReduce) completes under iteration i's matmuls.
            acc = small.tile([128, G_CHUNKS], f32)
            for j in range(G_CHUNKS):
                gsl = gpool.tile([128, G_CHUNK, NG], f32, tag="gsl")
                src = Wg[j * G_CHUNK * 128:(j + 1) * G_CHUNK * 128, :]
                geng = nc.sync if _rep == 0 else nc.scalar
                geng.dma_start(gsl[:], src.rearrange("(t p) c -> p t c", p=128))
                gscr = gpool.tile([128, G_CHUNK, NG], bf16, tag="gscr")
                nc.scalar.activation(
                    gscr[:], gsl[:], mybir.ActivationFunctionType.Abs,
                    accum_out=acc[:, j:j + 1])
            gpart = small.tile([128, 1], f32)
            gscr2 = small.tile([128, G_CHUNKS], bf16)
            nc.scalar.activation(
                gscr2[:], acc[:], mybir.ActivationFunctionType.Abs,
                accum_out=gpart[:])

            # ---- tiny AllReduce of per-partition partials ----
            gsum = small.tile([128, 1], f32)
            if ncores > 1 and use_collective:
                cin = dram.tile([128, 1], f32)
                nc.scalar.dma_start(cin[:], gpart[:])
                cout = dram.tile([128, 1], f32, tag="cout", name=f"cout{_rep}")
                nc.gpsimd.collective_compute(
                    "AllReduce", mybir.AluOpType.add,
                    replica_groups=[list(range(ncores))],
                    ins=[cin[:].opt()], outs=[cout[:].opt()])
                nc.scalar.dma_start(gsum[:], cout[:])
            else:
                # timing/TimelineSim variant: no collective (gamma from the
                # local shard only -- numerically wrong, timing-equivalent)
                nc.scalar.copy(gsum[:], gpart[:])

            # sum across partitions, result broadcast to all partitions
            gtot = small.tile([128, 1], f32)
            nc.gpsimd.partition_all_reduce(
                gtot[:], gsum[:], channels=128, reduce_op=bass_isa.ReduceOp.add)

            # threshold t = 0.5 * (gamma + 1e-5)
            # Wq = (w > t) - (w < -t)  in {-1, 0, 1}
            tsb = small.tile([128, 1], f32)
            nc.scalar.activation(
                tsb[:], gtot[:], mybir.ActivationFunctionType.Identity,
                bias=bias_p[:], scale=0.5 / N_ELEMS)
            ntsb = small.tile([128, 1], f32)
            nc.scalar.activation(
                ntsb[:], gtot[:], mybir.ActivationFunctionType.Identity,
                bias=bias_n[:], scale=-0.5 / N_ELEMS)

            # ---- main loop over output-feature chunks ----
            for c in range(NCHUNKS):
                # quantize this chunk's W half into DR pair layout
                wqd = wqdp.tile([128, NPAIR, 2, NCHUNK], f8, tag="wqd")
                for kt in range(KT):
                    wtmp = wpool.tile([128, NCHUNK], f32, tag="wtmp")
                    # W stream rides the ACT HWDGE ring: the sync ring's
                    # out-stores would head-of-line block next-chunk W
                    # prefetch. ACT's ring is otherwise only used by the
                    # early gamma work.
                    nc.scalar.dma_start(
                        wtmp[:], WT[ts(kt, 128), ts(c, NCHUNK)])
                    neg = spool.tile([128, NCHUNK], bf16, tag="neg")
                    nc.vector.tensor_scalar(
                        neg[:], wtmp[:], ntsb[:], None, mybir.AluOpType.is_lt)
                    if kt < P:
                        dsts = [wqd[:, kt, 0, :], wqd[:, kt, 1, :]]
                    else:
                        j = kt - P
                        dsts = [wqd[:, P + j // 2, j % 2, :]]
                    for dst in dsts:
                        nc.vector.scalar_tensor_tensor(
                            dst, wtmp[:], tsb[:], neg[:],
                            mybir.AluOpType.is_gt, mybir.AluOpType.subtract)

                # matmuls: stationary = wq pair (128 features x 2 subplanes),
                # moving = x pair (2 x MW tokens). One PSUM bank accumulates
                # full K per (nt, mh).
                for nt in range(NTILES):
                    ps = [pmain.tile([128, MW], f32, tag="ps",
                                     name=f"ps{nt % 2}_{mh}")
                          for mh in range(MH)]
                    n0 = nt * 128
                    for q in range(NPAIR):
                        lw = wqd[:, q, :, n0:n0 + 128]
                        for mh in range(MH):
                            nc.tensor.matmul(
                                ps[mh][:], lw,
                                xdr[:, q, :, ts(mh, MW)],
                                start=(q == 0), stop=(q == NPAIR - 1),
                                perf_mode=DR)
                    for mh in range(MH):
                        osb = opool.tile([128, MW], f32, tag="osb")
                        nc.vector.tensor_copy(osb[:], ps[mh][:])
                        nc.sync.dma_start(
                            outT[c * NCHUNK + n0:c * NCHUNK + n0 + 128,
                                 ts(mh, MW)], osb[:])

    nc.compile()
    meta = dict(m_core=m_core, k=k, n=n, ncores=ncores, NG=NG, kb=kb)
    return nc, meta


def _get_compiled():
    global _COMPILED
    if _COMPILED is None:
        _COMPILED = build_module(n_full=D_OUT)
    return _COMPILED


def make_in_maps(x, W, m_core=M_CORE, n_core=N_CORE, ncores=NCORES,
                 n_split=N_SPLIT, kb=KB_PLANES):
    """Host-side shard prep. x [B,S,D_IN] f32, W [D_OUT,D_IN] f32.
    Core c = (token-group c//n_split, n-half c%n_split).

    x is encoded as 48 e4m3-exact sub-planes stored in bf16, in DR pair
    order: pairs 0..P-1 are (hi, lo) of precise plane p; pairs P.. are
    (plane KB+2j, plane KB+2j+1) single-e4m3 cheap planes."""
    k = W.shape[1]
    n = W.shape[0]
    ng = n // ncores
    P = kb // 128
    x2 = np.asarray(x, dtype=np.float32).reshape(-1, k)
    f8 = ml_dtypes.float8_e4m3fn
    hi = x2.astype(f8).astype(np.float32)
    lo = (x2 - hi).astype(f8).astype(np.float32)
    # sub-plane stream [NSUB*128, tokens] built per token-group below
    planes = []
    for p in range(P):
        planes.append(hi[:, p * 128:(p + 1) * 128])
        planes.append(lo[:, p * 128:(p + 1) * 128])
    planes.append(hi[:, P * 128:])      # cheap planes, already e4m3-exact
    xs = np.concatenate(planes, axis=1).astype(ml_dtypes.bfloat16)
    WT = np.ascontiguousarray(np.asarray(W, dtype=np.float32).T)  # [k, n]
    xTg = [np.ascontiguousarray(xs[g * m_core:(g + 1) * m_core, :].T)
           for g in range(ncores // n_split)]
    in_maps = []
    for c in range(ncores):
        g, h = divmod(c, n_split)
        WTc = (WT if n_split == 1 else
               np.ascontiguousarray(WT[:, h * n_core:(h + 1) * n_core]))
        Wgc = np.ascontiguousarray(WT[:, c * ng:(c + 1) * ng])
        in_maps.append({"xT": xTg[g], "WT": WTc, "Wg": Wgc})
    return in_maps


def kernel(input, W):
    """Full inputs in, full output out. Shards internally across 8 cores."""
    global LAST_RESULTS
    from concourse import bass_utils

    nc, meta = _get_compiled()
    in_maps = make_in_maps(input, W)
    res = bass_utils.run_bass_kernel_spmd(
        nc, in_maps, core_ids=list(range(NCORES)))
    LAST_RESULTS = res
    rows = [np.concatenate([res.results[g * N_SPLIT + h]["outT"].T
                            for h in range(N_SPLIT)], axis=1)
            for g in range(M_GROUPS)]
    out = np.vstack(rows) if M_GROUPS > 1 else rows[0]
    return np.ascontiguousarray(out).reshape(B, S, D_OUT).astype(np.float32)


# revision 6
# speedup vs baseline: 1.0769x; 1.0764x over previous
"""BitLinear (ternary absmean-quantized linear) on 8 TRN2 NeuronCores.

Reference math (fp32):
    gamma = mean(|W|)
    Wq    = round(clip(W / (gamma + 1e-5), -1, 1))   # ternary {-1, 0, 1}
    out   = einsum('bsi,oi->bso', x, Wq)             # x @ Wq.T

Sharding: tokens x n-halves. x [4,2048,4096] -> 8192 tokens; cores are a
4x2 grid: token-group (2048 tokens) x out-feature half (2048 features).
Every core needs its W half; gamma (a global scalar) is computed
cooperatively: each core abs-sums 1/8 of W (512 of the 4096 columns of
WT), a tiny [128,1] AllReduce combines the partials, and each core then
quantizes its W half on the fly while the TensorEngine consumes it.

Precision/speed: ALL matmuls run as fp8 e4m3 DoubleRow pairs (one DR
pair-pass covers 2 K-planes at the same ns/row as one bf16 plane, and
measures slightly faster than the bf16/fp8 mix under the sustained-load
P0 clock of ~1.95 GHz; the kernel runs at the pure-PE roofline).
  - "precise" planes (first KB of K): an (hi, lo) DR pair per plane with
    hi = e4m3(x), lo = e4m3(x - hi) and the plane's ternary weights
    duplicated in both pair slots: contributes w*(hi+lo) ~ bf16-accurate.
  - "cheap" planes (rest): two K-planes per DR pair, single e4m3 each.
All fp8 values are pre-rounded to e4m3 on the HOST (RNE) and shipped as
bf16, so the device bf16->f8 copies are exact regardless of the DVE cast
rounding mode (this deployment's cast rounds worse than RNE).
Measured on the real inputs: fro rel err 1.88e-2, absmax-rel 1.94e-2.

Device kernel layout (per core), output TRANSPOSED (features x tokens):
    xT   [6144, 2048] bf16  - 48 e4m3-exact sub-planes (24 DR pairs), K-major
    WT   [4096, 2048] f32   - this core's W half, transposed (k major)
    Wg   [4096,  512] f32   - this core's gamma shard (512 columns of WT)
    outT [2048, 2048] f32   - out.T; host transposes back

Main loop: 4 N-chunks of 512 output features. Per chunk: stream 32 K-slabs
of WT f32 on the ACT HWDGE ring (the sync ring carries only out-stores, so
W prefetch is never head-of-line blocked), quantize each on DVE
((|w| > t) with sign, 2-3 ops) into a resident fp8 [128, 24, 2, 512] pair
buffer. Matmuls: per 128-feature n-tile, 24 DR pair-passes x 4 moving
tiles of 512 tokens accumulate full K in one PSUM bank.
"""

import numpy as np
import ml_dtypes

NCORES = 8

# Full-problem dims (hardcoded per the harness contract).
B, S, D_IN, D_OUT = 4, 2048, 4096, 4096
M_TOTAL = B * S            # 8192 tokens
N_SPLIT = 2                # n-halves (cores = 4 token groups x 2 halves)
M_GROUPS = NCORES // N_SPLIT
M_CORE = M_TOTAL // M_GROUPS   # tokens per core
N_CORE = D_OUT // N_SPLIT      # output features per core
KB_PLANES = 2048           # K-planes with hi+lo (precise); rest single-e4m3

_COMPILED = None   # cached (nc, meta)
LAST_RESULTS = None  # BassKernelResults of the most recent run (for test.py)


def build_module(m_core=M_CORE, k=D_IN, n=N_CORE, ncores=NCORES, repeat=1,
                 use_collective=True, kb=KB_PLANES, n_full=None,
                 wpool_bufs=8, stg_bufs=4, g_chunk=None):
    """Build + compile the SPMD Bass module. repeat>1 unrolls the whole
    kernel body multiple times inside one NEFF (for steady-state timing).

    n is this core's output-feature count; n_full (defaults to n) is the
    FULL problem's out_features, used only for the gamma normalization
    (gamma = sum|W| / (k * n_full), reduced across cores)."""
    if g_chunk is None:
        g_chunk = 4 if m_core <= 1024 else 2
    opool_bufs = 6 if m_core <= 1024 else 4
    spool_bufs = 6 if m_core <= 1024 else 4
    import concourse.bass as bass  # noqa: F401
    import concourse.mybir as mybir
    import concourse.tile as tile
    from concourse import bacc
    from concourse import bass_isa

    f32 = mybir.dt.float32
    bf16 = mybir.dt.bfloat16
    f8 = mybir.dt.float8e4
    KT = k // 128            # total k-tiles of 128
    P = kb // 128            # precise planes (hi+lo pair each)
    C = KT - P               # cheap planes (2 planes per pair)
    assert C % 2 == 0, "cheap plane count must be even"
    NPAIR = P + C // 2       # DR pair-passes per n-tile
    NSUB = 2 * NPAIR         # fp8 sub-planes in the x stream
    NCHUNK = min(512, n)     # output-feature chunk width
    NCHUNKS = n // NCHUNK
    NTILES = NCHUNK // 128   # n-tiles (stationary free dim) per chunk
    MW = min(512, m_core)    # moving-operand token width
    MH = m_core // MW        # moving tiles per pair-pass
    if n_full is None:
        n_full = n
    NG = n_full // 8         # gamma shard width (columns of full WT)
    G_CHUNK = min(g_chunk, KT)  # k-tiles per gamma reduce chunk
    G_CHUNKS = KT // G_CHUNK
    N_ELEMS = float(k * n_full)

    nc = bacc.Bacc("TRN2", target_bir_lowering=False, debug=False,
                   num_devices=ncores)
    xT = nc.dram_tensor("xT", [NSUB * 128, m_core], bf16,
                        kind="ExternalInput")
    WT = nc.dram_tensor("WT", [k, n], f32, kind="ExternalInput")
    Wg = nc.dram_tensor("Wg", [k, NG], f32, kind="ExternalInput")
    outT = nc.dram_tensor("outT", [n, m_core], f32, kind="ExternalOutput")

    ts = bass.ts
    DR = mybir.MatmulPerfMode.DoubleRow

    with tile.TileContext(nc) as tc:
        with (
            tc.tile_pool(name="xpool", bufs=1) as xpool,
            tc.tile_pool(name="xstg", bufs=stg_bufs) as xstg,
            tc.tile_pool(name="gpool", bufs=2) as gpool,
            tc.tile_pool(name="wqdp", bufs=2) as wqdp,
            tc.tile_pool(name="wpool", bufs=wpool_bufs) as wpool,
            tc.tile_pool(name="spool", bufs=spool_bufs) as spool,
            tc.tile_pool(name="opool", bufs=opool_bufs) as opool,
            tc.tile_pool(name="small", bufs=2) as small,
            tc.tile_pool(name="pmain", bufs=8, space="PSUM") as pmain,
            tc.tile_pool(name="dram", bufs=2, space="DRAM") as dram,
        ):
          with tc.tile_pool(name="cpool", bufs=1) as cpool:
            bias_p = cpool.tile([128, 1], f32, name="bias_p")
            nc.gpsimd.memset(bias_p[:], 0.5e-5)
            bias_n = cpool.tile([128, 1], f32, name="bias_n")
            nc.gpsimd.memset(bias_n[:], -0.5e-5)

          # ---- resident x: fp8 [128, NPAIR, 2, m] (hi/lo + cheap pairs) ----
          # Loaded once per NEFF execution (x does not change within one
          # launch). Host pre-rounds every sub-plane to an exact e4m3 value
          # stored in bf16; the DVE copy below is therefore exact.
          xdr = xpool.tile([128, NPAIR, 2, m_core], f8, name="xdr")
          xr = xT[:, :].rearrange("(t p) m -> p t m", p=128)
          for t in range(NSUB):
              stg = xstg.tile([128, m_core], bf16, tag="xstg")
              nc.sync.dma_start(stg[:], xr[:, t, :])
              nc.vector.tensor_copy(xdr[:, t // 2, t % 2, :], stg[:])

          for _rep in range(repeat):
            # ---- gamma: local abs-sum over this core's shard ----
            # On ACT + gpsimd queues, which run far ahead of the PE: in
            # steady state iteration i+1's whole gamma chain (including the
            # AllReduce) completes under iteration i's matmuls.
            acc = small.tile([128, G_CHUNKS], f32)
            for j in range(G_CHUNKS):
                gsl = gpool.tile([128, G_CHUNK, NG], f32, tag="gsl")
                src = Wg[j * G_CHUNK * 128:(j + 1) * G_CHUNK * 128, :]
                geng = nc.sync if _rep == 0 else nc.scalar
                geng.dma_start(gsl[:], src.rearrange("(t p) c -> p t c", p=128))
                gscr = gpool.tile([128, G_CHUNK, NG], bf16, tag="gscr")
                nc.scalar.activation(
                    gscr[:], gsl[:], mybir.ActivationFunctionType.Abs,
                    accum_out=acc[:, j:j + 1])
            gpart = small.tile([128, 1], f32)
            gscr2 = small.tile([128, G_CHUNKS], bf16)
            nc.scalar.activation(
                gscr2[:], acc[:], mybir.ActivationFunctionType.Abs,
                accum_out=gpart[:])

            # ---- tiny AllReduce of per-partition partials ----
            gsum = small.tile([128, 1], f32)
            if ncores > 1 and use_collective:
                cin = dram.tile([128, 1], f32)
                nc.scalar.dma_start(cin[:], gpart[:])
                cout = dram.tile([128, 1], f32, tag="cout", name=f"cout{_rep}")
                nc.gpsimd.collective_compute(
                    "AllReduce", mybir.AluOpType.add,
                    replica_groups=[list(range(ncores))],
                    ins=[cin[:].opt()], outs=[cout[:].opt()])
                nc.scalar.dma_start(gsum[:], cout[:])
            else:
                # timing/TimelineSim variant: no collective (gamma from the
                # local shard only -- numerically wrong, timing-equivalent)
                nc.scalar.copy(gsum[:], gpart[:])

            # sum across partitions, result broadcast to all partitions
            gtot = small.tile([128, 1], f32)
            nc.gpsimd.partition_all_reduce(
                gtot[:], gsum[:], channels=128, reduce_op=bass_isa.ReduceOp.add)

            # threshold t = 0.5 * (gamma + 1e-5)
            # Wq = (w > t) - (w < -t)  in {-1, 0, 1}
            tsb = small.tile([128, 1], f32)
            nc.scalar.activation(
                tsb[:], gtot[:], mybir.ActivationFunctionType.Identity,
                bias=bias_p[:], scale=0.5 / N_ELEMS)
            ntsb = small.tile([128, 1], f32)
            nc.scalar.activation(
                ntsb[:], gtot[:], mybir.ActivationFunctionType.Identity,
                bias=bias_n[:], scale=-0.5 / N_ELEMS)

            # ---- main loop over output-feature chunks ----
            for c in range(NCHUNKS):
                # quantize this chunk's W half into DR pair layout
                wqd = wqdp.tile([128, NPAIR, 2, NCHUNK], f8, tag="wqd")
                for kt in range(KT):
                    wtmp = wpool.tile([128, NCHUNK], f32, tag="wtmp")
                    # W stream rides the ACT HWDGE ring: the sync ring's
                    # out-stores would head-of-line block next-chunk W
                    # prefetch. ACT's ring is otherwise only used by the
                    # early gamma work.
                    nc.scalar.dma_start(
                        wtmp[:], WT[ts(kt, 128), ts(c, NCHUNK)])
                    neg = spool.tile([128, NCHUNK], bf16, tag="neg")
                    nc.vector.tensor_scalar(
                        neg[:], wtmp[:], ntsb[:], None, mybir.AluOpType.is_lt)
                    if kt < P:
                        dsts = [wqd[:, kt, 0, :], wqd[:, kt, 1, :]]
                    else:
                        j = kt - P
                        dsts = [wqd[:, P + j // 2, j % 2, :]]
                    for dst in dsts:
                        nc.vector.scalar_tensor_tensor(
                            dst, wtmp[:], tsb[:], neg[:],
                            mybir.AluOpType.is_gt, mybir.AluOpType.subtract)

                # matmuls: stationary = wq pair (128 features x 2 subplanes),
                # moving = x pair (2 x MW tokens). One PSUM bank accumulates
                # full K per (nt, mh).
                for nt in range(NTILES):
                    ps = [pmain.tile([128, MW], f32, tag="ps",
                                     name=f"ps{nt % 2}_{mh}")
                          for mh in range(MH)]
                    n0 = nt * 128
                    for q in range(NPAIR):
                        lw = wqd[:, q, :, n0:n0 + 128]
                        if q < P:
                            # precise pair: both DR sub-rows hold the SAME
                            # ternary weights, so load sub0 twice via a
                            # stride-0 AP (halves LDWEIGHTS SBUF reads;
                            # measured ~5-8 us/iter faster, walrus-clean)
                            lw = bass.AP(lw.tensor, lw.offset,
                                         [list(lw.ap[0]), [0, 2],
                                          list(lw.ap[2])])
                        for mh in range(MH):
                            nc.tensor.matmul(
                                ps[mh][:], lw,
                                xdr[:, q, :, ts(mh, MW)],
                                start=(q == 0), stop=(q == NPAIR - 1),
                                perf_mode=DR)
                    for mh in range(MH):
                        osb = opool.tile([128, MW], f32, tag="osb")
                        nc.vector.tensor_copy(osb[:], ps[mh][:])
                        nc.sync.dma_start(
                            outT[c * NCHUNK + n0:c * NCHUNK + n0 + 128,
                                 ts(mh, MW)], osb[:])

    nc.compile()
    meta = dict(m_core=m_core, k=k, n=n, ncores=ncores, NG=NG, kb=kb)
    return nc, meta


def _get_compiled():
    global _COMPILED
    if _COMPILED is None:
        _COMPILED = build_module(n_full=D_OUT)
    return _COMPILED


def make_in_maps(x, W, m_core=M_CORE, n_core=N_CORE, ncores=NCORES,
                 n_split=N_SPLIT, kb=KB_PLANES):
    """Host-side shard prep. x [B,S,D_IN] f32, W [D_OUT,D_IN] f32.
    Core c = (token-group c//n_split, n-half c%n_split).

    x is encoded as 48 e4m3-exact sub-planes stored in bf16, in DR pair
    order: pairs 0..P-1 are (hi, lo) of precise plane p; pairs P.. are
    (plane KB+2j, plane KB+2j+1) single-e4m3 cheap planes."""
    k = W.shape[1]
    n = W.shape[0]
    ng = n // ncores
    P = kb // 128
    x2 = np.asarray(x, dtype=np.float32).reshape(-1, k)
    f8 = ml_dtypes.float8_e4m3fn
    hi = x2.astype(f8).astype(np.float32)
    lo = (x2 - hi).astype(f8).astype(np.float32)
    # sub-plane stream [NSUB*128, tokens] built per token-group below
    planes = []
    for p in range(P):
        planes.append(hi[:, p * 128:(p + 1) * 128])
        planes.append(lo[:, p * 128:(p + 1) * 128])
    planes.append(hi[:, P * 128:])      # cheap planes, already e4m3-exact
    xs = np.concatenate(planes, axis=1).astype(ml_dtypes.bfloat16)
    WT = np.ascontiguousarray(np.asarray(W, dtype=np.float32).T)  # [k, n]
    xTg = [np.ascontiguousarray(xs[g * m_core:(g + 1) * m_core, :].T)
           for g in range(ncores // n_split)]
    in_maps = []
    for c in range(ncores):
        g, h = divmod(c, n_split)
        WTc = (WT if n_split == 1 else
               np.ascontiguousarray(WT[:, h * n_core:(h + 1) * n_core]))
        Wgc = np.ascontiguousarray(WT[:, c * ng:(c + 1) * ng])
        in_maps.append({"xT": xTg[g], "WT": WTc, "Wg": Wgc})
    return in_maps


def kernel(input, W):
    """Full inputs in, full output out. Shards internally across 8 cores."""
    global LAST_RESULTS
    from concourse import bass_utils

    nc, meta = _get_compiled()
    in_maps = make_in_maps(input, W)
    res = bass_utils.run_bass_kernel_spmd(
        nc, in_maps, core_ids=list(range(NCORES)))
    LAST_RESULTS = res
    rows = [np.concatenate([res.results[g * N_SPLIT + h]["outT"].T
                            for h in range(N_SPLIT)], axis=1)
            for g in range(M_GROUPS)]
    out = np.vstack(rows) if M_GROUPS > 1 else rows[0]
    return np.ascontiguousarray(out).reshape(B, S, D_OUT).astype(np.float32)


# revision 7
# speedup vs baseline: 1.1153x; 1.0357x over previous
"""BitLinear (ternary absmean-quantized linear) on 8 TRN2 NeuronCores.

Reference math (fp32):
    gamma = mean(|W|)
    Wq    = round(clip(W / (gamma + 1e-5), -1, 1))   # ternary {-1, 0, 1}
    out   = einsum('bsi,oi->bso', x, Wq)             # x @ Wq.T

Sharding: tokens x n-halves. x [4,2048,4096] -> 8192 tokens; cores are a
4x2 grid: token-group (2048 tokens) x out-feature half (2048 features).
Every core needs its W half; gamma (a global scalar) is computed
cooperatively: each core abs-sums 1/8 of W (512 of the 4096 columns of
WT), a tiny [128,1] AllReduce combines the partials, and each core then
quantizes its W half on the fly while the TensorEngine consumes it.

Precision/speed: ALL matmuls run as fp8 e4m3 DoubleRow pairs (one DR
pair-pass covers 2 K-planes at the same ns/row as one bf16 plane, and
measures slightly faster than the bf16/fp8 mix under the sustained-load
P0 clock of ~1.95 GHz; the kernel runs at the pure-PE roofline).
  - "precise" planes (first KB of K): an (hi, lo) DR pair per plane with
    hi = e4m3(x), lo = e4m3(x - hi) and the plane's ternary weights
    duplicated in both pair slots: contributes w*(hi+lo) ~ bf16-accurate.
  - "cheap" planes (rest): two K-planes per DR pair, single e4m3 each.
All fp8 values are pre-rounded to e4m3 on the HOST (RNE) and shipped as
bf16, so the device bf16->f8 copies are exact regardless of the DVE cast
rounding mode (this deployment's cast rounds worse than RNE).
Measured on the real inputs: fro rel err 1.88e-2, absmax-rel 1.94e-2.

Device kernel layout (per core), output TRANSPOSED (features x tokens):
    xT   [6144, 2048] bf16  - 48 e4m3-exact sub-planes (24 DR pairs), K-major
    WT   [4096, 2048] f32   - this core's W half, transposed (k major)
    Wg   [4096,  512] f32   - this core's gamma shard (512 columns of WT)
    outT [2048, 2048] f32   - out.T; host transposes back

Main loop: 4 N-chunks of 512 output features. Per chunk: stream 32 K-slabs
of WT f32 on the ACT HWDGE ring (the sync ring carries only out-stores, so
W prefetch is never head-of-line blocked), quantize each on DVE
((|w| > t) with sign, 2-3 ops) into a resident fp8 [128, 24, 2, 512] pair
buffer. Matmuls: per 128-feature n-tile, 24 DR pair-passes x 4 moving
tiles of 512 tokens accumulate full K in one PSUM bank.
"""

import numpy as np
import ml_dtypes

NCORES = 8

# Full-problem dims (hardcoded per the harness contract).
B, S, D_IN, D_OUT = 4, 2048, 4096, 4096
M_TOTAL = B * S            # 8192 tokens
N_SPLIT = 2                # n-halves (cores = 4 token groups x 2 halves)
M_GROUPS = NCORES // N_SPLIT
M_CORE = M_TOTAL // M_GROUPS   # tokens per core
N_CORE = D_OUT // N_SPLIT      # output features per core
KB_PLANES = 2048           # K-planes with hi+lo (precise); rest single-e4m3

_COMPILED = None   # cached (nc, meta)
LAST_RESULTS = None  # BassKernelResults of the most recent run (for test.py)


def build_module(m_core=M_CORE, k=D_IN, n=N_CORE, ncores=NCORES, repeat=1,
                 use_collective=True, kb=KB_PLANES, n_full=None,
                 wpool_bufs=8, stg_bufs=4, g_chunk=None):
    """Build + compile the SPMD Bass module. repeat>1 unrolls the whole
    kernel body multiple times inside one NEFF (for steady-state timing).

    n is this core's output-feature count; n_full (defaults to n) is the
    FULL problem's out_features, used only for the gamma normalization
    (gamma = sum|W| / (k * n_full), reduced across cores)."""
    if g_chunk is None:
        g_chunk = 4 if m_core <= 1024 else 2
    opool_bufs = 6 if m_core <= 1024 else 4
    spool_bufs = 6 if m_core <= 1024 else 4
    import concourse.bass as bass  # noqa: F401
    import concourse.mybir as mybir
    import concourse.tile as tile
    from concourse import bacc
    from concourse import bass_isa

    f32 = mybir.dt.float32
    bf16 = mybir.dt.bfloat16
    f8 = mybir.dt.float8e4
    KT = k // 128            # total k-tiles of 128
    P = kb // 128            # precise planes (hi+lo pair each)
    C = KT - P               # cheap planes (2 planes per pair)
    assert C % 2 == 0, "cheap plane count must be even"
    NPAIR = P + C // 2       # DR pair-passes per n-tile
    NSUB = 2 * NPAIR         # fp8 sub-planes in the x stream
    NCHUNK = min(512, n)     # output-feature chunk width
    NCHUNKS = n // NCHUNK
    NTILES = NCHUNK // 128   # n-tiles (stationary free dim) per chunk
    MW = min(512, m_core)    # moving-operand token width
    MH = m_core // MW        # moving tiles per pair-pass
    if n_full is None:
        n_full = n
    NG = n_full // 8         # gamma shard width (columns of full WT)
    G_CHUNK = min(g_chunk, KT)  # k-tiles per gamma reduce chunk
    G_CHUNKS = KT // G_CHUNK
    N_ELEMS = float(k * n_full)

    nc = bacc.Bacc("TRN2", target_bir_lowering=False, debug=False,
                   num_devices=ncores)
    xT = nc.dram_tensor("xT", [NSUB * 128, m_core], bf16,
                        kind="ExternalInput")
    WT = nc.dram_tensor("WT", [k, n], f32, kind="ExternalInput")
    Wg = nc.dram_tensor("Wg", [k, NG], f32, kind="ExternalInput")
    outT = nc.dram_tensor("outT", [n, m_core], f32, kind="ExternalOutput")

    ts = bass.ts
    DR = mybir.MatmulPerfMode.DoubleRow

    with tile.TileContext(nc) as tc:
        with (
            tc.tile_pool(name="xpool", bufs=1) as xpool,
            tc.tile_pool(name="xstg", bufs=stg_bufs) as xstg,
            tc.tile_pool(name="gpool", bufs=2) as gpool,
            tc.tile_pool(name="wqdp", bufs=2) as wqdp,
            tc.tile_pool(name="wpool", bufs=wpool_bufs) as wpool,
            tc.tile_pool(name="spool", bufs=spool_bufs) as spool,
            tc.tile_pool(name="opool", bufs=opool_bufs) as opool,
            tc.tile_pool(name="small", bufs=2) as small,
            tc.tile_pool(name="pmain", bufs=8, space="PSUM") as pmain,
            tc.tile_pool(name="dram", bufs=2, space="DRAM") as dram,
        ):
          with tc.tile_pool(name="cpool", bufs=1) as cpool:
            bias_p = cpool.tile([128, 1], f32, name="bias_p")
            nc.gpsimd.memset(bias_p[:], 0.5e-5)
            bias_n = cpool.tile([128, 1], f32, name="bias_n")
            nc.gpsimd.memset(bias_n[:], -0.5e-5)

          # ---- resident x: fp8 [128, NPAIR, 2, m] (hi/lo + cheap pairs) ----
          # Loaded once per NEFF execution (x does not change within one
          # launch). Host pre-rounds every sub-plane to an exact e4m3 value
          # stored in bf16; the DVE copy below is therefore exact.
          xdr = xpool.tile([128, NPAIR, 2, m_core], f8, name="xdr")
          xr = xT[:, :].rearrange("(t p) m -> p t m", p=128)
          for t in range(NSUB):
              stg = xstg.tile([128, m_core], bf16, tag="xstg")
              nc.sync.dma_start(stg[:], xr[:, t, :])
              nc.vector.tensor_copy(xdr[:, t // 2, t % 2, :], stg[:])

          for _rep in range(repeat):
            # ---- gamma: local abs-sum over this core's shard ----
            # On ACT + gpsimd queues, which run far ahead of the PE: in
            # steady state iteration i+1's whole gamma chain (including the
            # AllReduce) completes under iteration i's matmuls.
            acc = small.tile([128, G_CHUNKS], f32)
            for j in range(G_CHUNKS):
                gsl = gpool.tile([128, G_CHUNK, NG], f32, tag="gsl")
                src = Wg[j * G_CHUNK * 128:(j + 1) * G_CHUNK * 128, :]
                geng = nc.sync if _rep == 0 else nc.scalar
                geng.dma_start(gsl[:], src.rearrange("(t p) c -> p t c", p=128))
                gscr = gpool.tile([128, G_CHUNK, NG], bf16, tag="gscr")
                nc.scalar.activation(
                    gscr[:], gsl[:], mybir.ActivationFunctionType.Abs,
                    accum_out=acc[:, j:j + 1])
            gpart = small.tile([128, 1], f32)
            gscr2 = small.tile([128, G_CHUNKS], bf16)
            nc.scalar.activation(
                gscr2[:], acc[:], mybir.ActivationFunctionType.Abs,
                accum_out=gpart[:])

            # ---- tiny AllReduce of per-partition partials ----
            gsum = small.tile([128, 1], f32)
            if ncores > 1 and use_collective:
                cin = dram.tile([128, 1], f32)
                nc.scalar.dma_start(cin[:], gpart[:])
                cout = dram.tile([128, 1], f32, tag="cout", name=f"cout{_rep}")
                nc.gpsimd.collective_compute(
                    "AllReduce", mybir.AluOpType.add,
                    replica_groups=[list(range(ncores))],
                    ins=[cin[:].opt()], outs=[cout[:].opt()])
                nc.scalar.dma_start(gsum[:], cout[:])
            else:
                # timing/TimelineSim variant: no collective (gamma from the
                # local shard only -- numerically wrong, timing-equivalent)
                nc.scalar.copy(gsum[:], gpart[:])

            # sum across partitions, result broadcast to all partitions
            gtot = small.tile([128, 1], f32)
            nc.gpsimd.partition_all_reduce(
                gtot[:], gsum[:], channels=128, reduce_op=bass_isa.ReduceOp.add)

            # threshold t = 0.5 * (gamma + 1e-5)
            # Wq = (w > t) - (w < -t)  in {-1, 0, 1}
            tsb = small.tile([128, 1], f32)
            nc.scalar.activation(
                tsb[:], gtot[:], mybir.ActivationFunctionType.Identity,
                bias=bias_p[:], scale=0.5 / N_ELEMS)
            ntsb = small.tile([128, 1], f32)
            nc.scalar.activation(
                ntsb[:], gtot[:], mybir.ActivationFunctionType.Identity,
                bias=bias_n[:], scale=-0.5 / N_ELEMS)

            # ---- main loop over output-feature chunks ----
            for c in range(NCHUNKS):
                # quantize this chunk's W half into DR pair layout
                wqd = wqdp.tile([128, NPAIR, 2, NCHUNK], f8, tag="wqd")
                for kt in range(KT):
                    wtmp = wpool.tile([128, NCHUNK], f32, tag="wtmp")
                    # W stream rides the ACT HWDGE ring: the sync ring's
                    # out-stores would head-of-line block next-chunk W
                    # prefetch. ACT's ring is otherwise only used by the
                    # early gamma work.
                    nc.scalar.dma_start(
                        wtmp[:], WT[ts(kt, 128), ts(c, NCHUNK)])
                    neg = spool.tile([128, NCHUNK], bf16, tag="neg")
                    nc.vector.tensor_scalar(
                        neg[:], wtmp[:], ntsb[:], None, mybir.AluOpType.is_lt)
                    if kt < P:
                        # sub1 is never read (the stride-0 LDWEIGHTS AP
                        # loads sub0 into both DR rows), so write only sub0
                        dsts = [wqd[:, kt, 0, :]]
                    else:
                        j = kt - P
                        dsts = [wqd[:, P + j // 2, j % 2, :]]
                    for dst in dsts:
                        nc.vector.scalar_tensor_tensor(
                            dst, wtmp[:], tsb[:], neg[:],
                            mybir.AluOpType.is_gt, mybir.AluOpType.subtract)

                # matmuls: stationary = wq pair (128 features x 2 subplanes),
                # moving = x pair (2 x MW tokens). One PSUM bank accumulates
                # full K per (nt, mh).
                for nt in range(NTILES):
                    ps = [pmain.tile([128, MW], f32, tag="ps",
                                     name=f"ps{nt % 2}_{mh}")
                          for mh in range(MH)]
                    n0 = nt * 128
                    for q in range(NPAIR):
                        lw = wqd[:, q, :, n0:n0 + 128]
                        if q < P:
                            # precise pair: both DR sub-rows hold the SAME
                            # ternary weights, so load sub0 twice via a
                            # stride-0 AP (halves LDWEIGHTS SBUF reads;
                            # measured ~5-8 us/iter faster, walrus-clean)
                            lw = bass.AP(lw.tensor, lw.offset,
                                         [list(lw.ap[0]), [0, 2],
                                          list(lw.ap[2])])
                        for mh in range(MH):
                            nc.tensor.matmul(
                                ps[mh][:], lw,
                                xdr[:, q, :, ts(mh, MW)],
                                start=(q == 0), stop=(q == NPAIR - 1),
                                perf_mode=DR)
                    for mh in range(MH):
                        osb = opool.tile([128, MW], f32, tag="osb")
                        nc.vector.tensor_copy(osb[:], ps[mh][:])
                        nc.sync.dma_start(
                            outT[c * NCHUNK + n0:c * NCHUNK + n0 + 128,
                                 ts(mh, MW)], osb[:])

    nc.compile()
    meta = dict(m_core=m_core, k=k, n=n, ncores=ncores, NG=NG, kb=kb)
    return nc, meta


def _get_compiled():
    global _COMPILED
    if _COMPILED is None:
        _COMPILED = build_module(n_full=D_OUT)
    return _COMPILED


def make_in_maps(x, W, m_core=M_CORE, n_core=N_CORE, ncores=NCORES,
                 n_split=N_SPLIT, kb=KB_PLANES):
    """Host-side shard prep. x [B,S,D_IN] f32, W [D_OUT,D_IN] f32.
    Core c = (token-group c//n_split, n-half c%n_split).

    x is encoded as 48 e4m3-exact sub-planes stored in bf16, in DR pair
    order: pairs 0..P-1 are (hi, lo) of precise plane p; pairs P.. are
    (plane KB+2j, plane KB+2j+1) single-e4m3 cheap planes."""
    k = W.shape[1]
    n = W.shape[0]
    ng = n // ncores
    P = kb // 128
    x2 = np.asarray(x, dtype=np.float32).reshape(-1, k)
    f8 = ml_dtypes.float8_e4m3fn
    hi = x2.astype(f8).astype(np.float32)
    lo = (x2 - hi).astype(f8).astype(np.float32)
    # sub-plane stream [NSUB*128, tokens] built per token-group below
    planes = []
    for p in range(P):
        planes.append(hi[:, p * 128:(p + 1) * 128])
        planes.append(lo[:, p * 128:(p + 1) * 128])
    planes.append(hi[:, P * 128:])      # cheap planes, already e4m3-exact
    xs = np.concatenate(planes, axis=1).astype(ml_dtypes.bfloat16)
    WT = np.ascontiguousarray(np.asarray(W, dtype=np.float32).T)  # [k, n]
    xTg = [np.ascontiguousarray(xs[g * m_core:(g + 1) * m_core, :].T)
           for g in range(ncores // n_split)]
    in_maps = []
    for c in range(ncores):
        g, h = divmod(c, n_split)
        WTc = (WT if n_split == 1 else
               np.ascontiguousarray(WT[:, h * n_core:(h + 1) * n_core]))
        Wgc = np.ascontiguousarray(WT[:, c * ng:(c + 1) * ng])
        in_maps.append({"xT": xTg[g], "WT": WTc, "Wg": Wgc})
    return in_maps


def kernel(input, W):
    """Full inputs in, full output out. Shards internally across 8 cores."""
    global LAST_RESULTS
    from concourse import bass_utils

    nc, meta = _get_compiled()
    in_maps = make_in_maps(input, W)
    res = bass_utils.run_bass_kernel_spmd(
        nc, in_maps, core_ids=list(range(NCORES)))
    LAST_RESULTS = res
    rows = [np.concatenate([res.results[g * N_SPLIT + h]["outT"].T
                            for h in range(N_SPLIT)], axis=1)
            for g in range(M_GROUPS)]
    out = np.vstack(rows) if M_GROUPS > 1 else rows[0]
    return np.ascontiguousarray(out).reshape(B, S, D_OUT).astype(np.float32)
